# revision 52
# baseline (speedup 1.0000x reference)
"""MultiHeadAttention on 8 TRN2 NeuronCores.

Shapes (hardcoded): x [4, 2048, 1024], w_qkv [1024, 3072], b_qkv [3072],
w_o [1024, 1024], b_o [1024]; H=16 heads, head_dim=64, scale 1/8.

Sharding: core c -> batch c//2, head-group c%2 (8 heads each).
Each core computes its 8 heads' attention values and a partial o-proj
([2048, 1024] f32); host sums the two partials per batch, adds b_o and
the constant row bv @ w_o (v-bias folds out of attention since softmax
rows sum to 1).

V3 schedule: single-phase (projections/o-proj interleaved into the
attention steps), with
 - q/k/v projections in fp8e4 DoubleRow (K=256 per matmul). Weights are
   scaled x32 on the host so they sit in fp8's normal range; q/k scaling
   cancels via the exp scale (1/(32*32*8)), v scaling via the broadcast
   constant (1/32 instead of 1).
 - av matmuls in fp8e4 DoubleRow over key-chunk pairs (K=256); exp
   writes et directly as fp8 (values ~e^{N(0,1/9)} sit in fp8's normal
   range).
 - softmax reciprocal on DVE (reciprocal_approx_fast, full-partition:
   the custom op silently fails on base_partition != 0) instead of the
   scalar engine's ln/exp chain; keeps ACT free for the 256 exps and
   the step boundary stall-free (HAM stays warm).
 - deferred norm: each step's normalize chain runs early in the NEXT
   step; av issue slots are placed late enough to not block the
   in-order PE queue on the avps release.
"""

import os
import sys
import types

sys.path.insert(0, "/opt/trn_rl_repo")

import numpy as np
import ml_dtypes
from contextlib import ExitStack

import concourse.bass as bass  # noqa: F401
import concourse.tile as tile
from concourse import bacc, mybir
from concourse.bass_utils import run_bass_kernel_spmd

BF16 = mybir.dt.bfloat16
F32 = mybir.dt.float32
F8 = mybir.dt.float8e4
NBF = ml_dtypes.bfloat16
NF8 = ml_dtypes.float8_e4m3
DR = mybir.MatmulPerfMode.DoubleRow

N_CORES = 8
B, S, D, E = 4, 2048, 1024, 1024
H, HD = 16, 64
NP = 4    # head pairs per core
NQB = 4   # query blocks of 512
NKC = 16  # key/seq chunks of 128
NIC = 8   # input-dim chunks of 128

WS = 32.0           # host-side weight scale into fp8 normal range
EXP_SCALE = 1.0 / (WS * WS * 8.0)
AV_FP8 = os.environ.get("AV_FP8", "0") == "1"
PROJ_DR = os.environ.get("PROJ_DR", "0") == "1"

TRACE = os.environ.get("KERNEL_TRACE", "") == "1"
LAST_EXEC_NS = None

if os.environ.get("LDW_OPT", "0") == "1":
    # walrus is invoked with --enable-ldw-opt=false by default; flip it to
    # let the compiler elide/optimize redundant LDWEIGHTS
    import concourse.bass_utils as _bu
    if not getattr(_bu, "_ldw_patched", False):
        _orig_run_command = _bu.run_command

        def _patched_run_command(cmd, *a, **kw):
            cmd = ["--enable-ldw-opt=true" if c == "--enable-ldw-opt=false"
                   else c for c in cmd]
            return _orig_run_command(cmd, *a, **kw)

        _bu.run_command = _patched_run_command
        _bu._ldw_patched = True

if TRACE:
    _hook = [None]
    _ah = types.ModuleType("antenv.axon_hooks")
    _ah.set_axon_ntff_profile_hook = lambda h: _hook.__setitem__(0, h)
    _ah.get_axon_ntff_profile_hook = lambda: _hook[0]
    sys.modules["antenv.axon_hooks"] = _ah
    import antenv
    antenv.axon_hooks = _ah
    from trn_agent_boot.trn_boot import _ntff_profile_via_ctypes
    _ah.set_axon_ntff_profile_hook(
        _ntff_profile_via_ctypes("/opt/axon/libaxon_pjrt.so"))

_nc_cache = [None]


def _build():
    nc = bacc.Bacc("TRN2", target_bir_lowering=False, debug=False,
                   num_devices=N_CORES)
    XDT = F8 if PROJ_DR else BF16
    xT_ap = nc.dram_tensor("xT", [NIC, 128, S], XDT, kind="ExternalInput").ap()
    wq_ap = nc.dram_tensor("wq", [NIC, 128, 512], XDT, kind="ExternalInput").ap()
    wk_ap = nc.dram_tensor("wk", [NIC, 128, 512], XDT, kind="ExternalInput").ap()
    wv_ap = nc.dram_tensor("wv", [NIC, 128, 512], XDT, kind="ExternalInput").ap()
    wo_ap = nc.dram_tensor("wo", [NP, 128, 1024], BF16, kind="ExternalInput").ap()
    bq_ap = nc.dram_tensor("bq", [128, NP], F32, kind="ExternalInput").ap()
    bk_ap = nc.dram_tensor("bk", [128, NP], F32, kind="ExternalInput").ap()
    out_ap = nc.dram_tensor("out", [NKC, 128, 1024], F32,
                            kind="ExternalOutput").ap()

    with tile.TileContext(nc) as tc:
        with ExitStack() as ctx:
            sb = ctx.enter_context(tc.tile_pool(name="sb", bufs=1))
            xT_sb = sb.tile([128, NIC, S], XDT)
            wq_sb = sb.tile([128, NIC, 512], XDT)
            wk_sb = sb.tile([128, NIC, 512], XDT)
            wv_sb = sb.tile([128, NIC, 512], XDT)
            wo_sb = sb.tile([128, NP, 1024], BF16)
            bq_sb = sb.tile([128, NP], F32)
            bk_sb = sb.tile([128, NP], F32)
            # v (x32 scale) per (kc-pair, head, kc-parity); 80-col pad keeps
            # the DoubleRow Ko stride 16B-aligned; col 64 stays 1.0 for the
            # softmax denominator row.
            if AV_FP8:
                v_aug = sb.tile([128, NKC // 2, 8, 2, 80], F8)
            else:
                v_aug = sb.tile([128, NKC, 8, 65], BF16)

            # zero-padded q: [:, 0, :] holds head A on partitions 0:64
            # (zeros below), [:, 1, :] holds head B on partitions 64:128
            # (zeros above). Lets qk run as ONE K=128 N=1024 matmul in the
            # default 128x128 PE mode — the auto-inferred 64-row tiling of
            # the two K=64 matmuls forced a PE mode-switch drain against
            # the 128x128 av/proj matmuls at every transition.
            # layout [128, qb, 2, 512]: the two padded halves are contiguous
            # per query block so the qk moving AP merges to 2D [128, 1024]
            qT = [sb.tile([128, NQB, 2, 512], BF16, name=f"qT{p}")
                  for p in range(NP)]
            kT = [sb.tile([128, S], BF16, name=f"kT{p}") for p in range(NP)]
            valsT = [sb.tile([128, S], BF16, name=f"valsT{p}")
                     for p in range(NP)]

            for ic in range(NIC):
                nc.sync.dma_start(out=xT_sb[:, ic, :], in_=xT_ap[ic])
                nc.sync.dma_start(out=wv_sb[:, ic, :], in_=wv_ap[ic])
            for ic in range(NIC):
                nc.sync.dma_start(out=wk_sb[:, ic, :], in_=wk_ap[ic])
                nc.sync.dma_start(out=wq_sb[:, ic, :], in_=wq_ap[ic])
            nc.sync.dma_start(out=bq_sb[:], in_=bq_ap[:])
            nc.sync.dma_start(out=bk_sb[:], in_=bk_ap[:])
            for p in range(NP):
                nc.sync.dma_start(out=wo_sb[:, p, :], in_=wo_ap[p])
            # ones column at WS: denominator comes out as WS*sum(et), so
            # 1/WS of the v unscale folds into the reciprocal for free
            nc.gpsimd.memset(v_aug[:], WS)
            for j in range(NP):
                nc.gpsimd.memset(qT[j][:], 0.0)

            qkps = ctx.enter_context(
                tc.tile_pool(name="qkps", bufs=2, space="PSUM"))
            avps = ctx.enter_context(
                tc.tile_pool(name="avps", bufs=1, space="PSUM"))
            accs = ctx.enter_context(
                tc.tile_pool(name="accs", bufs=2, space="PSUM"))
            # deep enough that no et buffer is reused before av(0) of the
            # step has run (av(0) is gated on the previous step's norm
            # chain releasing the single-buffered av PSUM)
            eps = ctx.enter_context(tc.tile_pool(name="eps", bufs=8))
            rbs = ctx.enter_context(tc.tile_pool(name="rbs", bufs=2))
            ost = ctx.enter_context(tc.tile_pool(name="ost", bufs=2))

            def vproj_group(kc):
                acc = accs.tile([128, 512], F32, name="acc")
                if PROJ_DR:
                    for ic in range(0, NIC, 2):
                        nc.tensor.matmul(
                            acc[:],
                            xT_sb[:, ic:ic + 2, kc * 128:(kc + 1) * 128],
                            wv_sb[:, ic:ic + 2, :],
                            start=(ic == 0), stop=(ic == NIC - 2),
                            perf_mode=DR)
                else:
                    for ic in range(NIC):
                        nc.tensor.matmul(
                            acc[:], xT_sb[:, ic, kc * 128:(kc + 1) * 128],
                            wv_sb[:, ic, :],
                            start=(ic == 0), stop=(ic == NIC - 1))
                if AV_FP8:
                    nc.vector.tensor_copy(
                        v_aug[:, kc // 2, :, kc % 2, 0:64], acc[:])
                else:
                    nc.vector.tensor_copy(v_aug[:, kc, :, 0:64], acc[:])

            def proj_mms(which, p, qb):
                """One closure per matmul so fillers can spread across the
                qk/exp stream instead of head-blocking the in-order PE
                queue with an 8-matmul burst."""
                qcols = slice(qb * 512, (qb + 1) * 512)
                pcols = slice(p * 128, (p + 1) * 128)
                w_sb, b_sb, dst = ((wq_sb, bq_sb, qT) if which == "q"
                                   else (wk_sb, bk_sb, kT))
                state = {}

                def mk(ic):
                    def one():
                        if ic == 0:
                            state["acc"] = accs.tile([128, 512], F32,
                                                     name="acc")
                        acc = state["acc"]
                        nc.tensor.matmul(
                            acc[:], w_sb[:, ic, pcols], xT_sb[:, ic, qcols],
                            start=(ic == 0), stop=(ic == NIC - 1))
                        if ic == NIC - 1:
                            if which == "q":
                                nc.vector.tensor_scalar_add(
                                    dst[p][0:64, qb, 0, :], acc[0:64, :],
                                    b_sb[0:64, p:p + 1])
                                nc.vector.tensor_scalar_add(
                                    dst[p][64:128, qb, 1, :], acc[64:128, :],
                                    b_sb[64:128, p:p + 1])
                            else:
                                nc.vector.tensor_scalar_add(
                                    dst[p][:, qcols], acc[:],
                                    b_sb[:, p:p + 1])
                    return one
                return [mk(ic) for ic in range(NIC)]

            def proj_group(which, p, qb):
                for f in proj_mms(which, p, qb):
                    f()

            def oproj_mms(sc):
                scols = slice(sc * 128, (sc + 1) * 128)
                state = {}

                def mk(half, p):
                    hcols = slice(half * 512, (half + 1) * 512)

                    def one():
                        if half == 0 and p == 0:
                            state["stage"] = ost.tile([128, 1024], F32,
                                                      name="ostage")
                        if p == 0:
                            state["og"] = accs.tile([128, 512], F32,
                                                    name="acc")
                        og = state["og"]
                        nc.tensor.matmul(
                            og[:], valsT[p][:, scols], wo_sb[:, p, hcols],
                            start=(p == 0), stop=(p == NP - 1))
                        if p == NP - 1:
                            nc.vector.tensor_copy(
                                state["stage"][:, hcols], og[:])
                            if half == 1:
                                nc.sync.dma_start(out=out_ap[sc],
                                                  in_=state["stage"][:])
                    return one
                return [mk(h, p) for h in range(2) for p in range(NP)]

            def oproj_sc(sc):
                for f in oproj_mms(sc):
                    f()

            pend_norm = [None]

            def make_norm(p, qb, av, recip):
                qcols = slice(qb * 512, (qb + 1) * 512)

                def norm():
                    # broadcast the reciprocal row to 64 partitions on the
                    # otherwise-idle GPSIMD engine (no PE matmul, no PSUM)
                    bc_sb = rbs.tile([128, 1024], BF16, name="bcsb")
                    nc.gpsimd.partition_broadcast(
                        bc_sb[0:64, 0:1024], recip[0:1, 0:1024])
                    nc.vector.tensor_mul(
                        valsT[p][0:64, qcols], av[0:64, 0:512],
                        bc_sb[0:64, 0:512])
                    nc.vector.tensor_mul(
                        valsT[p][64:128, qcols], av[0:64, 512:1024],
                        bc_sb[0:64, 512:1024])
                return norm

            def attention_step(p, qb, fillers, oproj_fill=False):
                qcols = slice(qb * 512, (qb + 1) * 512)
                etps = {}

                def qk(kc):
                    kcols = slice(kc * 128, (kc + 1) * 128)
                    slot = qkps.tile([128, 1024], F32, name="qkslot")
                    # two K=128 N=512 matmuls (PSUM bank limit), both full
                    # mode at tile (0,0) with the same kT stationary: the
                    # zero halves of qT kill the cross-head terms
                    nc.tensor.matmul(
                        slot[:, 0:512], kT[p][:, kcols],
                        qT[p][:, qb, 0, :], start=True, stop=True)
                    nc.tensor.matmul(
                        slot[:, 512:1024], kT[p][:, kcols],
                        qT[p][:, qb, 1, :], start=True, stop=True)
                    if AV_FP8:
                        if kc % 2 == 0:
                            etps[kc // 2] = eps.tile([128, 2, 1024], F8,
                                                     name="etp")
                        dst = etps[kc // 2][:, kc % 2, :]
                    else:
                        etps[kc] = eps.tile([128, 1024], BF16, name="etp")
                        dst = etps[kc][:, :]
                    nc.scalar.activation(
                        dst, slot[:],
                        mybir.ActivationFunctionType.Exp, scale=EXP_SCALE)

                av_t = avps.tile([128, 1024], F32, name="av")

                def av(j):
                    if AV_FP8:
                        etp = etps.pop(j)
                        for h in range(2):
                            hc = slice(h * 512, (h + 1) * 512)
                            nc.tensor.matmul(
                                av_t[0:65, hc],
                                v_aug[:, j, 2 * p + h, :, 0:65],
                                etp[:, :, hc],
                                start=(j == 0), stop=(j == NKC // 2 - 1),
                                perf_mode=DR)
                    else:
                        for kc in (2 * j, 2 * j + 1):
                            et = etps[kc] if kc < NKC - 1 else etps.pop(kc)
                            for h in range(2):
                                hc = slice(h * 512, (h + 1) * 512)
                                nc.tensor.matmul(
                                    av_t[0:65, hc],
                                    v_aug[:, kc, 2 * p + h, :],
                                    et[:, hc],
                                    start=(kc == 0), stop=(kc == NKC - 1))

                # issue slots: av(j) late enough that (a) exp(2j+1) is done,
                # (b) av(0) does not block the in-order PE queue on the
                # previous step's norm muls (avps release); norm at qk(4) so
                # its chain doesn't stall the queue on the DVE recip.
                # fillers: ONE matmul per slot so the qk stream (and hence
                # the exp stream) is never head-blocked by a filler burst.
                av_after = {6: 0, 7: 1, 8: 2, 9: 3, 10: 4, 12: 5, 14: 6}
                queue = list(fillers)
                # o-proj fillers read valsT written by this step's
                # pend_norm muls — don't drain them before kc5
                drain_from = 5 if oproj_fill else 1

                def drain(k):
                    for _ in range(min(k, len(queue))):
                        queue.pop(0)()

                for kc in range(NKC):
                    qk(kc)
                    if kc == 0 and pend_norm[0] is not None:
                        # norm is DVE/GPSIMD-only: issue ASAP so the avps
                        # release (its muls) lands before av(0) at kc4
                        pend_norm[0]()
                    if kc in av_after:
                        av(av_after[kc])
                    if kc >= drain_from:
                        free_slot = kc not in av_after and kc > 4
                        rate = ((3 if free_slot else 2) if oproj_fill
                                else (2 if free_slot else 1))
                        drain(rate)
                drain(len(queue))
                av(NKC // 2 - 1)
                # full-partition op: the custom DVE recip silently fails on
                # base_partition != 0; rows other than 64 are discarded
                recip_f = rbs.tile([128, 1024], F32, name="recipf")
                nc.vector.reciprocal_approx_fast(
                    recip_f[:, 0:1024], av_t[:, 0:1024])
                # cross-partition (64 -> 0) downcast copy; partition 0
                # feeds partition_broadcast. The 1/WS unscale is already in
                # the reciprocal via the WS-valued ones column.
                recip = rbs.tile([128, 1024], BF16, name="recip")
                nc.vector.tensor_copy(
                    recip[0:1, 0:1024], recip_f[64:65, 0:1024])
                pend_norm[0] = make_norm(p, qb, av_t, recip)

            # ---- prefix: v-proj (all kc) + q/k-proj for pair 0 ----
            for kc in range(NKC):
                vproj_group(kc)
            for qb in range(NQB):
                proj_group("q", 0, qb)
                proj_group("k", 0, qb)

            # ---- attention steps with interleaved proj / o-proj ----
            for i in range(NP * NQB):
                p, qb = i // NQB, i % NQB
                if i <= 11:
                    pn, j = i // 4 + 1, i % 4
                    fillers = (proj_mms("q", pn, j) + proj_mms("k", pn, j))
                    opf = False
                elif i == 12:
                    fillers = []
                    opf = False
                else:
                    blk = i - 13
                    fillers = [f for sc in range(4 * blk, 4 * blk + 4)
                               for f in oproj_mms(sc)]
                    opf = True
                attention_step(p, qb, fillers, oproj_fill=opf)

            # ---- tail: last norm + last o-proj block ----
            pend_norm[0]()
            for sc in range(12, 16):
                oproj_sc(sc)

    nc.compile()
    return nc


def kernel(x, w_qkv, b_qkv, w_o, b_o):
    global LAST_EXEC_NS
    if _nc_cache[0] is None:
        _nc_cache[0] = _build()
    nc = _nc_cache[0]

    NXDT = NF8 if PROJ_DR else NBF
    xT_b = [np.ascontiguousarray(x[b].T).astype(NXDT).reshape(NIC, 128, S)
            for b in range(B)]
    w = w_qkv.astype(np.float32) * WS
    bqkv = b_qkv.astype(np.float32) * WS
    in_maps = []
    for c in range(N_CORES):
        b, g = c // 2, c % 2
        # reference packs qkv per head: head h -> cols [h*192, (h+1)*192),
        # q dims 0:64, k 64:128, v 128:192 within
        heads = np.arange(g * 8, g * 8 + 8)
        qs = (heads[:, None] * 192 + np.arange(64)).ravel()
        ks = (heads[:, None] * 192 + 64 + np.arange(64)).ravel()
        vs = (heads[:, None] * 192 + 128 + np.arange(64)).ravel()
        in_maps.append({
            "xT": xT_b[b],
            "wq": w[:, qs].astype(NXDT).reshape(NIC, 128, 512),
            "wk": w[:, ks].astype(NXDT).reshape(NIC, 128, 512),
            "wv": w[:, vs].astype(NXDT).reshape(NIC, 128, 512),
            "wo": w_o[g * 512:(g + 1) * 512, :].astype(NBF).reshape(
                NP, 128, 1024),
            "bq": np.ascontiguousarray(bqkv[qs].reshape(NP, 128).T),
            "bk": np.ascontiguousarray(bqkv[ks].reshape(NP, 128).T),
        })

    res = run_bass_kernel_spmd(nc, in_maps, list(range(N_CORES)),
                               trace=TRACE)
    LAST_EXEC_NS = res.exec_time_ns

    # v-bias folds out of attention: softmax rows sum to 1, so
    # vals_h = p_h @ (x W_vh) + b_vh and the b_vh term contributes the
    # constant row (concat_h b_vh) @ w_o
    vs_full = (np.arange(H)[:, None] * 192 + 128 + np.arange(HD)).ravel()
    bvwo = b_qkv[vs_full].astype(np.float32) @ w_o.astype(np.float32)

    out = np.empty((B, S, E), np.float32)
    bias = b_o.astype(np.float32) + bvwo
    for b in range(B):
        p0 = np.asarray(res.results[2 * b]["out"],
                        np.float32).reshape(S, E)
        p1 = np.asarray(res.results[2 * b + 1]["out"],
                        np.float32).reshape(S, E)
        out[b] = p0 + p1 + bias
    return out


# revision 54
# speedup vs baseline: 1.1558x; 1.1558x over previous
"""MultiHeadAttention on 8 TRN2 NeuronCores.

Shapes (hardcoded): x [4, 2048, 1024], w_qkv [1024, 3072], b_qkv [3072],
w_o [1024, 1024], b_o [1024]; H=16 heads, head_dim=64, scale 1/8.

Sharding: core c -> batch c//2, head-group c%2 (8 heads each).
Each core computes its 8 heads' attention values and a partial o-proj
([2048, 1024] f32); host sums the two partials per batch, adds b_o and
the constant row bv @ w_o (v-bias folds out of attention since softmax
rows sum to 1).

V3 schedule: single-phase (projections/o-proj interleaved into the
attention steps), with
 - q/k/v projections in fp8e4 DoubleRow (K=256 per matmul). Weights are
   scaled x32 on the host so they sit in fp8's normal range; q/k scaling
   cancels via the exp scale (1/(32*32*8)), v scaling via the broadcast
   constant (1/32 instead of 1).
 - av matmuls in fp8e4 DoubleRow over key-chunk pairs (K=256); exp
   writes et directly as fp8 (values ~e^{N(0,1/9)} sit in fp8's normal
   range).
 - softmax reciprocal on DVE (reciprocal_approx_fast, full-partition:
   the custom op silently fails on base_partition != 0) instead of the
   scalar engine's ln/exp chain; keeps ACT free for the 256 exps and
   the step boundary stall-free (HAM stays warm).
 - deferred norm: each step's normalize chain runs early in the NEXT
   step; av issue slots are placed late enough to not block the
   in-order PE queue on the avps release.
"""

import os
import sys
import types

sys.path.insert(0, "/opt/trn_rl_repo")

import numpy as np
import ml_dtypes
from contextlib import ExitStack

import concourse.bass as bass  # noqa: F401
import concourse.tile as tile
from concourse import bacc, mybir
from concourse.bass_utils import run_bass_kernel_spmd

BF16 = mybir.dt.bfloat16
F32 = mybir.dt.float32
F8 = mybir.dt.float8e4
NBF = ml_dtypes.bfloat16
NF8 = ml_dtypes.float8_e4m3
DR = mybir.MatmulPerfMode.DoubleRow

N_CORES = 8
B, S, D, E = 4, 2048, 1024, 1024
H, HD = 16, 64
NP = 4    # head pairs per core
NQB = 4   # query blocks of 512
NKC = 16  # key/seq chunks of 128
NIC = 8   # input-dim chunks of 128

WS = 32.0           # host-side weight scale into fp8 normal range
EXP_SCALE = 1.0 / (WS * WS * 8.0)
AV_FP8 = os.environ.get("AV_FP8", "0") == "1"
PROJ_DR = os.environ.get("PROJ_DR", "0") == "1"

TRACE = os.environ.get("KERNEL_TRACE", "") == "1"
LAST_EXEC_NS = None

if os.environ.get("LDW_OPT", "0") == "1":
    # walrus is invoked with --enable-ldw-opt=false by default; flip it to
    # let the compiler elide/optimize redundant LDWEIGHTS
    import concourse.bass_utils as _bu
    if not getattr(_bu, "_ldw_patched", False):
        _orig_run_command = _bu.run_command

        def _patched_run_command(cmd, *a, **kw):
            cmd = ["--enable-ldw-opt=true" if c == "--enable-ldw-opt=false"
                   else c for c in cmd]
            return _orig_run_command(cmd, *a, **kw)

        _bu.run_command = _patched_run_command
        _bu._ldw_patched = True

if TRACE:
    _hook = [None]
    _ah = types.ModuleType("antenv.axon_hooks")
    _ah.set_axon_ntff_profile_hook = lambda h: _hook.__setitem__(0, h)
    _ah.get_axon_ntff_profile_hook = lambda: _hook[0]
    sys.modules["antenv.axon_hooks"] = _ah
    import antenv
    antenv.axon_hooks = _ah
    from trn_agent_boot.trn_boot import _ntff_profile_via_ctypes
    _ah.set_axon_ntff_profile_hook(
        _ntff_profile_via_ctypes("/opt/axon/libaxon_pjrt.so"))

_nc_cache = [None]


def _build():
    nc = bacc.Bacc("TRN2", target_bir_lowering=False, debug=False,
                   num_devices=N_CORES)
    XDT = F8 if PROJ_DR else BF16
    xT_ap = nc.dram_tensor("xT", [NIC, 128, S], XDT, kind="ExternalInput").ap()
    wq_ap = nc.dram_tensor("wq", [NIC, 128, 512], XDT, kind="ExternalInput").ap()
    wk_ap = nc.dram_tensor("wk", [NIC, 128, 512], XDT, kind="ExternalInput").ap()
    wv_ap = nc.dram_tensor("wv", [NIC, 128, 512], XDT, kind="ExternalInput").ap()
    wo_ap = nc.dram_tensor("wo", [NP, 128, 1024], BF16, kind="ExternalInput").ap()
    bq_ap = nc.dram_tensor("bq", [128, NP], F32, kind="ExternalInput").ap()
    bk_ap = nc.dram_tensor("bk", [128, NP], F32, kind="ExternalInput").ap()
    out_ap = nc.dram_tensor("out", [NKC, 128, 1024], F32,
                            kind="ExternalOutput").ap()

    with tile.TileContext(nc) as tc:
        with ExitStack() as ctx:
            sb = ctx.enter_context(tc.tile_pool(name="sb", bufs=1))
            xT_sb = sb.tile([128, NIC, S], XDT)
            wq_sb = sb.tile([128, NIC, 512], XDT)
            wk_sb = sb.tile([128, NIC, 512], XDT)
            wv_sb = sb.tile([128, NIC, 512], XDT)
            wo_sb = sb.tile([128, NP, 1024], BF16)
            bq_sb = sb.tile([128, NP], F32)
            bk_sb = sb.tile([128, NP], F32)
            # v (x32 scale) per (kc-pair, head, kc-parity); 80-col pad keeps
            # the DoubleRow Ko stride 16B-aligned; col 64 stays 1.0 for the
            # softmax denominator row.
            if AV_FP8:
                v_aug = sb.tile([128, NKC // 2, 8, 2, 80], F8)
            else:
                v_aug = sb.tile([128, NKC, 8, 65], BF16)

            # zero-padded q: [:, 0, :] holds head A on partitions 0:64
            # (zeros below), [:, 1, :] holds head B on partitions 64:128
            # (zeros above). Lets qk run as ONE K=128 N=1024 matmul in the
            # default 128x128 PE mode — the auto-inferred 64-row tiling of
            # the two K=64 matmuls forced a PE mode-switch drain against
            # the 128x128 av/proj matmuls at every transition.
            # layout [128, qb, 2, 512]: the two padded halves are contiguous
            # per query block so the qk moving AP merges to 2D [128, 1024]
            qT = [sb.tile([128, NQB, 2, 512], BF16, name=f"qT{p}")
                  for p in range(NP)]
            kT = [sb.tile([128, S], BF16, name=f"kT{p}") for p in range(NP)]
            valsT = [sb.tile([128, S], BF16, name=f"valsT{p}")
                     for p in range(NP)]

            for ic in range(NIC):
                nc.sync.dma_start(out=xT_sb[:, ic, :], in_=xT_ap[ic])
                nc.sync.dma_start(out=wv_sb[:, ic, :], in_=wv_ap[ic])
            for ic in range(NIC):
                nc.sync.dma_start(out=wk_sb[:, ic, :], in_=wk_ap[ic])
                nc.sync.dma_start(out=wq_sb[:, ic, :], in_=wq_ap[ic])
            nc.sync.dma_start(out=bq_sb[:], in_=bq_ap[:])
            nc.sync.dma_start(out=bk_sb[:], in_=bk_ap[:])
            for p in range(NP):
                nc.sync.dma_start(out=wo_sb[:, p, :], in_=wo_ap[p])
            # ones column at WS: denominator comes out as WS*sum(et), so
            # 1/WS of the v unscale folds into the reciprocal for free
            nc.gpsimd.memset(v_aug[:], WS)
            for j in range(NP):
                nc.gpsimd.memset(qT[j][:], 0.0)

            qkps = ctx.enter_context(
                tc.tile_pool(name="qkps", bufs=2, space="PSUM"))
            avps = ctx.enter_context(
                tc.tile_pool(name="avps", bufs=1, space="PSUM"))
            accs = ctx.enter_context(
                tc.tile_pool(name="accs", bufs=2, space="PSUM"))
            eps = ctx.enter_context(tc.tile_pool(name="eps", bufs=4))
            rbs = ctx.enter_context(tc.tile_pool(name="rbs", bufs=2))
            ost = ctx.enter_context(tc.tile_pool(name="ost", bufs=2))

            def vproj_group(kc):
                acc = accs.tile([128, 512], F32, name="acc")
                if PROJ_DR:
                    for ic in range(0, NIC, 2):
                        nc.tensor.matmul(
                            acc[:],
                            xT_sb[:, ic:ic + 2, kc * 128:(kc + 1) * 128],
                            wv_sb[:, ic:ic + 2, :],
                            start=(ic == 0), stop=(ic == NIC - 2),
                            perf_mode=DR)
                else:
                    for ic in range(NIC):
                        nc.tensor.matmul(
                            acc[:], xT_sb[:, ic, kc * 128:(kc + 1) * 128],
                            wv_sb[:, ic, :],
                            start=(ic == 0), stop=(ic == NIC - 1))
                if AV_FP8:
                    nc.vector.tensor_copy(
                        v_aug[:, kc // 2, :, kc % 2, 0:64], acc[:])
                else:
                    nc.vector.tensor_copy(v_aug[:, kc, :, 0:64], acc[:])

            def proj_mms(which, p, qb):
                """One closure per matmul so fillers can spread across the
                qk/exp stream instead of head-blocking the in-order PE
                queue with an 8-matmul burst."""
                qcols = slice(qb * 512, (qb + 1) * 512)
                pcols = slice(p * 128, (p + 1) * 128)
                w_sb, b_sb, dst = ((wq_sb, bq_sb, qT) if which == "q"
                                   else (wk_sb, bk_sb, kT))
                state = {}

                def mk(ic):
                    def one():
                        if ic == 0:
                            state["acc"] = accs.tile([128, 512], F32,
                                                     name="acc")
                        acc = state["acc"]
                        nc.tensor.matmul(
                            acc[:], w_sb[:, ic, pcols], xT_sb[:, ic, qcols],
                            start=(ic == 0), stop=(ic == NIC - 1))
                        if ic == NIC - 1:
                            if which == "q":
                                nc.vector.tensor_scalar_add(
                                    dst[p][0:64, qb, 0, :], acc[0:64, :],
                                    b_sb[0:64, p:p + 1])
                                nc.vector.tensor_scalar_add(
                                    dst[p][64:128, qb, 1, :], acc[64:128, :],
                                    b_sb[64:128, p:p + 1])
                            else:
                                nc.vector.tensor_scalar_add(
                                    dst[p][:, qcols], acc[:],
                                    b_sb[:, p:p + 1])
                    return one
                return [mk(ic) for ic in range(NIC)]

            def proj_group(which, p, qb):
                for f in proj_mms(which, p, qb):
                    f()

            def oproj_mms(sc):
                scols = slice(sc * 128, (sc + 1) * 128)
                state = {}

                def mk(half, p):
                    hcols = slice(half * 512, (half + 1) * 512)

                    def one():
                        if half == 0 and p == 0:
                            state["stage"] = ost.tile([128, 1024], F32,
                                                      name="ostage")
                        if p == 0:
                            state["og"] = accs.tile([128, 512], F32,
                                                    name="acc")
                        og = state["og"]
                        nc.tensor.matmul(
                            og[:], valsT[p][:, scols], wo_sb[:, p, hcols],
                            start=(p == 0), stop=(p == NP - 1))
                        if p == NP - 1:
                            nc.vector.tensor_copy(
                                state["stage"][:, hcols], og[:])
                            if half == 1:
                                nc.sync.dma_start(out=out_ap[sc],
                                                  in_=state["stage"][:])
                    return one
                return [mk(h, p) for h in range(2) for p in range(NP)]

            def oproj_sc(sc):
                for f in oproj_mms(sc):
                    f()

            pend_norm = [None]

            def make_norm(p, qb, av, recip):
                qcols = slice(qb * 512, (qb + 1) * 512)

                def norm():
                    # broadcast the reciprocal row to 64 partitions on the
                    # otherwise-idle GPSIMD engine (no PE matmul, no PSUM)
                    bc_sb = rbs.tile([128, 1024], BF16, name="bcsb")
                    nc.gpsimd.partition_broadcast(
                        bc_sb[0:64, 0:1024], recip[0:1, 0:1024])
                    nc.vector.tensor_mul(
                        valsT[p][0:64, qcols], av[0:64, 0:512],
                        bc_sb[0:64, 0:512])
                    nc.vector.tensor_mul(
                        valsT[p][64:128, qcols], av[0:64, 512:1024],
                        bc_sb[0:64, 512:1024])
                return norm

            def attention_step(p, qb, fillers, oproj_fill=False):
                qcols = slice(qb * 512, (qb + 1) * 512)
                etps = {}

                def qk(kc):
                    kcols = slice(kc * 128, (kc + 1) * 128)
                    slot = qkps.tile([128, 1024], F32, name="qkslot")
                    # two K=128 N=512 matmuls (PSUM bank limit), both full
                    # mode at tile (0,0) with the same kT stationary: the
                    # zero halves of qT kill the cross-head terms
                    nc.tensor.matmul(
                        slot[:, 0:512], kT[p][:, kcols],
                        qT[p][:, qb, 0, :], start=True, stop=True)
                    nc.tensor.matmul(
                        slot[:, 512:1024], kT[p][:, kcols],
                        qT[p][:, qb, 1, :], start=True, stop=True)
                    if AV_FP8:
                        if kc % 2 == 0:
                            etps[kc // 2] = eps.tile([128, 2, 1024], F8,
                                                     name="etp")
                        dst = etps[kc // 2][:, kc % 2, :]
                    else:
                        etps[kc] = eps.tile([128, 1024], BF16, name="etp")
                        dst = etps[kc][:, :]
                    nc.scalar.activation(
                        dst, slot[:],
                        mybir.ActivationFunctionType.Exp, scale=EXP_SCALE)

                av_t = avps.tile([128, 1024], F32, name="av")

                def av(j):
                    if AV_FP8:
                        etp = etps.pop(j)
                        for h in range(2):
                            hc = slice(h * 512, (h + 1) * 512)
                            nc.tensor.matmul(
                                av_t[0:65, hc],
                                v_aug[:, j, 2 * p + h, :, 0:65],
                                etp[:, :, hc],
                                start=(j == 0), stop=(j == NKC // 2 - 1),
                                perf_mode=DR)
                    else:
                        for kc in (2 * j, 2 * j + 1):
                            et = etps[kc] if kc < NKC - 1 else etps.pop(kc)
                            for h in range(2):
                                hc = slice(h * 512, (h + 1) * 512)
                                nc.tensor.matmul(
                                    av_t[0:65, hc],
                                    v_aug[:, kc, 2 * p + h, :],
                                    et[:, hc],
                                    start=(kc == 0), stop=(kc == NKC - 1))

                # issue slots: av(j) late enough that (a) exp(2j+1) is done,
                # (b) av(0) does not block the in-order PE queue on the
                # previous step's norm muls (avps release); norm at qk(4) so
                # its chain doesn't stall the queue on the DVE recip.
                # fillers: ONE matmul per slot so the qk stream (and hence
                # the exp stream) is never head-blocked by a filler burst.
                av_after = {6: 0, 7: 1, 8: 2, 9: 3, 10: 4, 12: 5, 14: 6}
                queue = list(fillers)
                # o-proj fillers read valsT written by this step's
                # pend_norm muls — don't drain them before kc5
                drain_from = 5 if oproj_fill else 1

                def drain(k):
                    for _ in range(min(k, len(queue))):
                        queue.pop(0)()

                for kc in range(NKC):
                    qk(kc)
                    if kc == 4 and pend_norm[0] is not None:
                        pend_norm[0]()
                    if kc in av_after:
                        av(av_after[kc])
                    if kc >= drain_from:
                        free_slot = kc not in av_after and kc > 4
                        rate = ((4 if free_slot else 2) if oproj_fill
                                else (2 if free_slot else 1))
                        drain(rate)
                drain(len(queue))
                av(NKC // 2 - 1)
                # full-partition op: the custom DVE recip silently fails on
                # base_partition != 0; rows other than 64 are discarded
                recip_f = rbs.tile([128, 1024], F32, name="recipf")
                nc.vector.reciprocal_approx_fast(
                    recip_f[:, 0:1024], av_t[:, 0:1024])
                # cross-partition (64 -> 0) downcast copy; partition 0
                # feeds partition_broadcast. The 1/WS unscale is already in
                # the reciprocal via the WS-valued ones column.
                recip = rbs.tile([128, 1024], BF16, name="recip")
                nc.vector.tensor_copy(
                    recip[0:1, 0:1024], recip_f[64:65, 0:1024])
                pend_norm[0] = make_norm(p, qb, av_t, recip)

            # ---- prefix: v-proj (all kc) + q/k-proj for pair 0 ----
            for kc in range(NKC):
                vproj_group(kc)
            for qb in range(NQB):
                proj_group("q", 0, qb)
                proj_group("k", 0, qb)

            # ---- attention steps with interleaved proj / o-proj ----
            for i in range(NP * NQB):
                p, qb = i // NQB, i % NQB
                if i <= 11:
                    pn, j = i // 4 + 1, i % 4
                    fillers = (proj_mms("q", pn, j) + proj_mms("k", pn, j))
                    opf = False
                elif i == 12:
                    fillers = []
                    opf = False
                else:
                    blk = i - 13
                    fillers = [f for sc in range(4 * blk, 4 * blk + 4)
                               for f in oproj_mms(sc)]
                    opf = True
                attention_step(p, qb, fillers, oproj_fill=opf)

            # ---- tail: last norm + last o-proj block ----
            pend_norm[0]()
            for sc in range(12, 16):
                oproj_sc(sc)

    nc.compile()
    return nc


def kernel(x, w_qkv, b_qkv, w_o, b_o):
    global LAST_EXEC_NS
    if _nc_cache[0] is None:
        _nc_cache[0] = _build()
    nc = _nc_cache[0]

    NXDT = NF8 if PROJ_DR else NBF
    xT_b = [np.ascontiguousarray(x[b].T).astype(NXDT).reshape(NIC, 128, S)
            for b in range(B)]
    w = w_qkv.astype(np.float32) * WS
    bqkv = b_qkv.astype(np.float32) * WS
    in_maps = []
    for c in range(N_CORES):
        b, g = c // 2, c % 2
        # reference packs qkv per head: head h -> cols [h*192, (h+1)*192),
        # q dims 0:64, k 64:128, v 128:192 within
        heads = np.arange(g * 8, g * 8 + 8)
        qs = (heads[:, None] * 192 + np.arange(64)).ravel()
        ks = (heads[:, None] * 192 + 64 + np.arange(64)).ravel()
        vs = (heads[:, None] * 192 + 128 + np.arange(64)).ravel()
        in_maps.append({
            "xT": xT_b[b],
            "wq": w[:, qs].astype(NXDT).reshape(NIC, 128, 512),
            "wk": w[:, ks].astype(NXDT).reshape(NIC, 128, 512),
            "wv": w[:, vs].astype(NXDT).reshape(NIC, 128, 512),
            "wo": w_o[g * 512:(g + 1) * 512, :].astype(NBF).reshape(
                NP, 128, 1024),
            "bq": np.ascontiguousarray(bqkv[qs].reshape(NP, 128).T),
            "bk": np.ascontiguousarray(bqkv[ks].reshape(NP, 128).T),
        })

    res = run_bass_kernel_spmd(nc, in_maps, list(range(N_CORES)),
                               trace=TRACE)
    LAST_EXEC_NS = res.exec_time_ns

    # v-bias folds out of attention: softmax rows sum to 1, so
    # vals_h = p_h @ (x W_vh) + b_vh and the b_vh term contributes the
    # constant row (concat_h b_vh) @ w_o
    vs_full = (np.arange(H)[:, None] * 192 + 128 + np.arange(HD)).ravel()
    bvwo = b_qkv[vs_full].astype(np.float32) @ w_o.astype(np.float32)

    out = np.empty((B, S, E), np.float32)
    bias = b_o.astype(np.float32) + bvwo
    for b in range(B):
        p0 = np.asarray(res.results[2 * b]["out"],
                        np.float32).reshape(S, E)
        p1 = np.asarray(res.results[2 * b + 1]["out"],
                        np.float32).reshape(S, E)
        out[b] = p0 + p1 + bias
    return out
